# revision 11
# baseline (speedup 1.0000x reference)
"""ALNN layer kernel for 8 TRN2 NeuronCores (raw Bass, explicit semaphores).

out[b,r,d] = relu( sum_l w_v[r,l,d]*relu(z[b,r,l,d]) + L*b_v[r,d] )
z = wt0*X + wt1*relu(X)*k + wt2*M + wt3*PD + 4*bt
k = exp(-relu(alpha_r)*|T - s_r|)        (uses relu(X*k) == relu(X)*k, k>0)

Sharding: B split 2 ways x R dealt into 4 buckets -> 8 cores, 16 b x 12 r
each. Cores c and c+4 share bucket c%4. Within a bucket the r's with
relu(alpha)=0 come first; for those, k == 1 so ABS/EXP/g are skipped
(q = relu(X)*wt1 directly). The graph is SPMD-uniform: the fast-path
iteration count is min over buckets of their zero count, so every core
runs the same instruction stream (extra zeros just take the slow path,
which is still correct since exp(-0*dist) == 1).

Per-core layout: partitions = L(128), free = (b=16, d=64) = 1024.

v3 engine plan (v2 profiling: all three compute engines ~balanced at
~3.3us/iter but kernel 68.5us vs DVE busy 45.8us -> the win is pipeline
fill + DMA: bt4 was host-expanded 16x to [L,1024] per r, 3.1MB/core of
the 5.6MB DMA, pacing the whole kernel):
 - DVE: the 4-5 weighted products per iter + wl (bf16 2x mode), plus the
   final PSUM->ob adds. ~41us busy = the roofline for this kernel shape.
 - PE:  z accumulated in PSUM: bt4 reconstructed from a 16KB transposed
   copy via a one-hot-d matmul (kills the bt4x DMA), then q/m0/m2/m3 via
   identity matmuls; final L-reduction as one-hot-column matmuls.
 - ACT: dist=abs, k=exp (slow iters only), lat=relu(PSUM z), final relu.
 - All inputs DMA'd up front (2.7MB/core), ordered so X/W arrive first;
   DVE starts ~3us in.

Raw bass: this toolchain's walrus allows at most ONE attached sync-wait
per compute instruction, so cross-engine deps use standalone wait_ge
instructions; DMA completion uses dedicated semaphores per stream.
"""

import os
import numpy as np
import ml_dtypes

import concourse.bass as bass
import concourse.mybir as mybir
from concourse.bass_utils import run_bass_kernel_spmd

AF = mybir.ActivationFunctionType
OP = mybir.AluOpType
BF16 = mybir.dt.bfloat16
F32 = mybir.dt.float32

B, R, L, D = 32, 48, 128, 64
NB, NK = 2, 4              # b-halves x r-buckets = 8 cores
BC, RC = B // NB, R // NK  # 16 b, 12 r per core
FD = BC * D                # 1024 free elems

# packed f32 const layout: [Tt 1024 | Abc 12 | NASbc 12]
CF_W = FD + RC + RC
# packed bf16 const layout: [Xt | Mt | PDt | OH 144 | I 128]
CB_W = 3 * FD + RC * RC + L
WS_W = 5 * D               # per-iter param slice: [wt0|wt1|wt2|wt3|wv]

_nbf16 = ml_dtypes.bfloat16

LB = 5   # wl buffers (DVE -> PE)
LAB = 4  # lat buffers (ACT -> DVE)
PRB = 3  # product buffer sets (DVE -> PE)
ZB = 3   # psum z triple-buffer (6 of 8 banks; ps0/ps1 take the rest)
WLAG = 3  # wl(i-WLAG) emitted in DVE iter i
ALAG = 2  # lat(i-ALAG) emitted in ACT iter i
NWARM = 16  # PE warmup matmuls (keep PE out of low p-state until work)


def _dve_schedule(nfast):
    """DVE emission order; must match the @block.vector body exactly."""
    sched = [("xp", 0)]
    for i in range(RC):
        if i >= WLAG:
            sched.append(("wl", i - WLAG))
        if i >= nfast:
            sched.append(("g", i))
        sched.append(("q", i))
        sched.append(("m0", i))
        sched.append(("m2", i))
        sched.append(("m3", i))
    for rr in range(RC - WLAG, RC - 2):
        sched.append(("wl", rr))
    sched.append(("wlp", RC - 2))
    sched.append(("wlp", RC - 1))
    return sched


def _build_graph(nfast, detect_races=True):
    nslow = RC - nfast
    ksl = max(nslow, 1)
    sched = _dve_schedule(nfast)
    C = {key: idx + 1 for idx, key in enumerate(sched)}

    nc = bass.Bass(detect_race_conditions=detect_races)
    cf_e = nc.declare_dram_parameter("cf", [L, CF_W], F32, isOutput=False)
    cb_e = nc.declare_dram_parameter("cb", [L, CB_W], BF16, isOutput=False)
    W_e = nc.declare_dram_parameter("W", [L, RC * WS_W], BF16, isOutput=False)
    bt_e = nc.declare_dram_parameter("BT", [L, RC * L + 16], BF16, isOutput=False)
    oh_e = nc.declare_dram_parameter("OHD", [L, 512], BF16, isOutput=False)
    out_e = nc.declare_dram_parameter("out", [RC, FD], F32, isOutput=True)

    from contextlib import ExitStack

    with ExitStack() as ctx:
        e = ctx.enter_context
        cf = e(nc.sbuf_tensor([L, CF_W], F32))
        cb = e(nc.sbuf_tensor([L, CB_W], BF16))
        Wb = e(nc.sbuf_tensor([L, RC * WS_W], BF16))
        BT = e(nc.sbuf_tensor([L, RC * L + 16], BF16))
        OHD = e(nc.sbuf_tensor([L, 512], BF16))
        Xp = e(nc.sbuf_tensor([L, FD], BF16))
        dist = e(nc.sbuf_tensor([L, FD], F32))
        kbuf = e(nc.sbuf_tensor([L, ksl * FD], BF16))
        g = e(nc.sbuf_tensor([L, FD], BF16))
        qb = e(nc.sbuf_tensor([L, PRB * FD], BF16))
        m0b = e(nc.sbuf_tensor([L, PRB * FD], BF16))
        m2b = e(nc.sbuf_tensor([L, PRB * FD], BF16))
        m3b = e(nc.sbuf_tensor([L, PRB * FD], BF16))
        latb = e(nc.sbuf_tensor([L, LAB * FD], BF16))
        wlbuf = e(nc.sbuf_tensor([L, LB * FD], BF16))
        outt = e(nc.sbuf_tensor([RC, FD], F32))
        wsc = e(nc.sbuf_tensor([L, 512], BF16))
        psz = [e(nc.psum_tensor(f"psz{j}", [L, FD], F32)) for j in range(ZB)]
        ps0 = e(nc.psum_tensor([RC, 512], F32))
        ps1 = e(nc.psum_tensor([RC, 512], F32))
        cfsem = e(nc.semaphore("cfsem"))
        cbsem = e(nc.semaphore("cbsem"))    # cb X part
        cbmsem = e(nc.semaphore("cbmsem"))  # cb M part
        cbpsem = e(nc.semaphore("cbpsem"))  # cb PD part
        cbtsem = e(nc.semaphore("cbtsem"))  # cb OH3+Ident part
        wsem0 = e(nc.semaphore("wsem0"))    # W iter 0
        wsema = e(nc.semaphore("wsema"))    # W iters 1-5
        wsemb = e(nc.semaphore("wsemb"))    # W iters 6-11
        ohsem = e(nc.semaphore("ohsem"))    # OHD
        btsem = e(nc.semaphore("btsem"))    # BT
        asem = e(nc.semaphore("asem"))      # ACT k completions
        lsem = e(nc.semaphore("lsem"))      # ACT lat/final relu completions
        zsem = e(nc.semaphore("zsem"))      # PE z-group completions (1/iter)
        msem = e(nc.semaphore("msem"))      # PE out-mm completions (1/iter)
        vsem = e(nc.semaphore("vsem"))      # DVE op completions
        osem = e(nc.semaphore("osem"))
        gsem = e(nc.semaphore("gsem"))
        block = e(nc.Block())

        Tt = cf[:, 0:FD]
        Abc = cf[:, FD : FD + RC]
        NASbc = cf[:, FD + RC : FD + 2 * RC]
        Xt = cb[:, 0:FD]
        Mt = cb[:, FD : 2 * FD]
        PDt = cb[:, 2 * FD : 3 * FD]
        OH3 = cb[:, 3 * FD : 3 * FD + RC * RC].rearrange("p (r m) -> p r m", r=RC)
        Ident = cb[:, 3 * FD + RC * RC :]

        def r3(ap):
            return ap.rearrange("p (b d) -> p b d", b=BC)

        def kslot(j):
            return kbuf[:, (j % ksl) * FD : (j % ksl + 1) * FD]

        def wbc(i, ei):
            base = i * WS_W + ei * D
            return Wb[:, base : base + D].unsqueeze(1).broadcast_to([L, BC, D])

        def btsl(i):
            return BT[0:64, i * L : (i + 1) * L]

        lbvT = BT[0:64, RC * L : RC * L + RC]

        def latslot(rr):
            return latb[:, (rr % LAB) * FD : (rr % LAB + 1) * FD]

        def wlslot(rr):
            return wlbuf[:, (rr % LB) * FD : (rr % LB + 1) * FD]

        def prod(buf, i):
            return buf[:, (i % PRB) * FD : (i % PRB + 1) * FD]

        @block.sync
        def _(sp):
            sp.dma_start(out=cb[:, 0:FD], in_=cb_e[:, 0:FD]).then_inc(cbsem, 16)
            sp.dma_start(out=Wb[:, 0:WS_W], in_=W_e[:, 0:WS_W]).then_inc(wsem0, 16)
            sp.dma_start(out=cb[:, FD : 2 * FD], in_=cb_e[:, FD : 2 * FD]).then_inc(
                cbmsem, 16
            )
            sp.dma_start(
                out=cb[:, 2 * FD : 3 * FD], in_=cb_e[:, 2 * FD : 3 * FD]
            ).then_inc(cbpsem, 16)
            sp.dma_start(out=cb[:, 3 * FD :], in_=cb_e[:, 3 * FD :]).then_inc(
                cbtsem, 16
            )
            sp.dma_start(
                out=Wb[:, WS_W : 6 * WS_W], in_=W_e[:, WS_W : 6 * WS_W]
            ).then_inc(wsema, 16)
            sp.dma_start(out=OHD[:, :], in_=oh_e[:, :]).then_inc(ohsem, 16)
            sp.dma_start(out=BT[:, :], in_=bt_e[:, :]).then_inc(btsem, 16)
            sp.dma_start(
                out=Wb[:, 6 * WS_W :], in_=W_e[:, 6 * WS_W :]
            ).then_inc(wsemb, 16)
            sp.wait_ge(lsem, 11)
            sp.dma_start(out=out_e[:, 0:512], in_=outt[:, 0:512]).then_inc(osem, 16)
            sp.wait_ge(lsem, 12)
            sp.dma_start(out=out_e[:, 512:], in_=outt[:, 512:]).then_inc(osem, 16)

        @block.scalar
        def _(act):
            act.dma_start(out=cf[:, :], in_=cf_e[:, :]).then_inc(cfsem, 16)
            act.wait_ge(cfsem, 16)
            for i in range(RC):
                # relus rr=0..9 only; last two z's go through the DVE wlp path
                if ALAG <= i and i - ALAG <= RC - WLAG:
                    rr = i - ALAG
                    act.wait_ge(zsem, rr + 1)
                    if rr >= LAB:
                        act.wait_ge(vsem, C[("wl", rr - LAB)])
                    nc.scalar.activation(
                        latslot(rr), psz[rr % ZB][:, :], AF.Relu
                    ).then_inc(lsem, 1)
                if i < nslow:
                    si = nfast + i
                    nc.scalar.activation(
                        dist[:, :], Tt, AF.Abs,
                        bias=NASbc[:, si : si + 1], scale=Abc[:, si : si + 1],
                    )
                    nc.scalar.activation(
                        kslot(i), dist[:, :], AF.Exp, scale=-1.0
                    ).then_inc(asem, 1)
            act.wait_ge(msem, 13)
            nc.scalar.activation(outt[:, 0:512], ps0[:, :], AF.Relu).then_inc(lsem, 1)
            nc.scalar.activation(outt[:, 512:], ps1[:, :], AF.Relu).then_inc(lsem, 1)

        @block.vector
        def _(ve):
            ve.wait_ge(cbsem, 16)
            nc.vector.tensor_scalar_max(Xp[:, :], Xt, 0.0).then_inc(vsem, 1)
            for i in range(RC):
                if i >= WLAG:
                    rr = i - WLAG
                    ve.wait_ge(lsem, rr + 1)
                    if rr >= LB:
                        ve.wait_ge(msem, rr - LB + 1)
                    nc.vector.tensor_tensor(
                        r3(wlslot(rr)), r3(latslot(rr)), wbc(rr, 4), OP.mult
                    ).then_inc(vsem, 1)
                if i >= nfast:
                    ve.wait_ge(asem, i - nfast + 1)
                    nc.vector.tensor_mul(g[:, :], Xp[:, :], kslot(i - nfast)).then_inc(
                        vsem, 1
                    )
                    qsrc = g[:, :]
                else:
                    qsrc = Xp[:, :]
                if i == 0:
                    ve.wait_ge(wsem0, 16)
                elif i == 1:
                    ve.wait_ge(wsema, 16)
                elif i == 6:
                    ve.wait_ge(wsemb, 16)
                nc.vector.tensor_tensor(
                    r3(prod(qb, i)), r3(qsrc), wbc(i, 1), OP.mult
                ).then_inc(vsem, 1)
                nc.vector.tensor_tensor(
                    r3(prod(m0b, i)), r3(Xt), wbc(i, 0), OP.mult
                ).then_inc(vsem, 1)
                if i == 0:
                    ve.wait_ge(cbmsem, 16)
                nc.vector.tensor_tensor(
                    r3(prod(m2b, i)), r3(Mt), wbc(i, 2), OP.mult
                ).then_inc(vsem, 1)
                if i == 0:
                    ve.wait_ge(cbpsem, 16)
                nc.vector.tensor_tensor(
                    r3(prod(m3b, i)), r3(PDt), wbc(i, 3), OP.mult
                ).then_inc(vsem, 1)
            for rr in range(RC - WLAG, RC - 2):
                ve.wait_ge(lsem, rr + 1)
                ve.wait_ge(msem, rr - LB + 1)
                nc.vector.tensor_tensor(
                    r3(wlslot(rr)), r3(latslot(rr)), wbc(rr, 4), OP.mult
                ).then_inc(vsem, 1)
            for rr in (RC - 2, RC - 1):
                # fused relu*wv straight from PSUM: skips the ACT round-trip
                # on the drain-critical last two iterations
                ve.wait_ge(zsem, rr + 1)
                ve.wait_ge(msem, rr - LB + 1)
                nc.vector.scalar_tensor_tensor(
                    r3(wlslot(rr)), r3(psz[rr % ZB][:, :]), 0.0, wbc(rr, 4),
                    OP.max, OP.mult,
                ).then_inc(vsem, 1)

        @block.gpsimd
        def _(gp):
            nc.gpsimd.memset(wsc[:, :], 1.0).then_inc(gsem, 1)

        @block.tensor
        def _(te):
            # warmup: keep the PE out of its low p-state until real work
            # arrives (~4.5us in). Results never read; ps0 reset by the
            # real start=True.
            te.wait_ge(gsem, 1)
            for _w in range(NWARM):
                nc.tensor.matmul(
                    ps0[:, :], wsc[:, 0:RC], wsc[:, :],
                    start=True, stop=True, skip_group_check=True,
                )
            te.wait_ge(ohsem, 16)
            te.wait_ge(btsem, 16)
            te.wait_ge(cbtsem, 16)
            for i in range(RC):
                te.wait_ge(vsem, C[("m3", i)])
                if i >= ZB:
                    te.wait_ge(lsem, i - ZB + 1)
                pz = psz[i % ZB]
                for pb, first in ((qb, True), (m0b, False), (m2b, False), (m3b, False)):
                    for h in range(2):
                        c0, c1 = h * 512, (h + 1) * 512
                        nc.tensor.matmul(
                            pz[:, c0:c1], Ident, prod(pb, i)[:, c0:c1],
                            start=first, stop=False, skip_group_check=True,
                        )
                for h in range(2):
                    mm = nc.tensor.matmul(
                        pz[:, h * 512 : (h + 1) * 512], btsl(i), OHD[0:64, :],
                        start=False, stop=True, skip_group_check=True,
                    )
                    if h == 1:
                        mm.then_inc(zsem, 1)
                if i >= WLAG:
                    rr = i - WLAG
                    te.wait_ge(vsem, C[("wl", rr)])
                    wl = wlslot(rr)
                    nc.tensor.matmul(
                        ps0[:, :], OH3[:, rr, :], wl[:, 0:512],
                        start=(rr == 0), stop=False, skip_group_check=True,
                    )
                    nc.tensor.matmul(
                        ps1[:, :], OH3[:, rr, :], wl[:, 512:1024],
                        start=(rr == 0), stop=False, skip_group_check=True,
                    ).then_inc(msem, 1)
            for rr in range(RC - WLAG, RC):
                key = ("wl", rr) if rr <= RC - WLAG else ("wlp", rr)
                te.wait_ge(vsem, C[key])
                wl = wlslot(rr)
                nc.tensor.matmul(
                    ps0[:, :], OH3[:, rr, :], wl[:, 0:512],
                    start=False, stop=False, skip_group_check=True,
                )
                nc.tensor.matmul(
                    ps1[:, :], OH3[:, rr, :], wl[:, 512:1024],
                    start=False, stop=False, skip_group_check=True,
                ).then_inc(msem, 1)
            # accumulate the L*b_v bias rows, then close both groups
            nc.tensor.matmul(
                ps0[:, :], lbvT, OHD[0:64, 0:512],
                start=False, stop=True, skip_group_check=True,
            )
            nc.tensor.matmul(
                ps1[:, :], lbvT, OHD[0:64, 0:512],
                start=False, stop=True, skip_group_check=True,
            ).then_inc(msem, 1)

    return nc


_CACHE = {}


def _buckets(a):
    """Deal r-indices into NK buckets of RC, zeros-first in each bucket.
    Returns (buckets, nfast): nfast = min zero-count across buckets."""
    zeros = [r for r in range(R) if a[r] == 0.0]
    pos = [r for r in range(R) if a[r] != 0.0]
    buckets = [[] for _ in range(NK)]
    for j, r in enumerate(zeros):
        buckets[j % NK].append(r)
    zc = [len(b) for b in buckets]
    pi = 0
    for k in range(NK):
        while len(buckets[k]) < RC:
            buckets[k].append(pos[pi])
            pi += 1
    nfast = min(min(zc), RC)
    return buckets, nfast


def _prepare(X, T, M, PD, alpha, w_v, w_t, b_t, b_v, ref_time):
    """Pack full inputs into per-core DRAM parameter maps.
    Returns (nfast, buckets, in_maps)."""
    a = np.maximum(alpha.reshape(R), 0.0)
    s_ref = ref_time.reshape(R)
    nas = -(a * s_ref)
    bt4 = 4.0 * b_t[..., 0]              # [R, L, D]
    lbv = float(L) * b_v[:, 0, :]        # [R, D]

    buckets, nfast = _buckets(a)

    # per-r params: [wt0|wt1|wt2|wt3|wv] (5*D per iter)
    wts = np.stack(
        [w_t[..., 0], w_t[..., 1], w_t[..., 2], w_t[..., 3], w_v], axis=2
    )                                     # [R, L, 5, D]

    oh = np.zeros((L, RC, RC), np.float32)
    for r in range(RC):
        oh[:, r, r] = 1.0
    ident = np.eye(L, dtype=np.float32)
    ohd = np.zeros((L, 512), np.float32)
    for b in range(8):
        for d in range(64):
            ohd[d, b * 64 + d] = 1.0

    in_maps = []
    for c in range(8):
        b0 = (c // NK) * BC
        rl = buckets[c % NK]
        tr = lambda x: np.ascontiguousarray(
            x[b0 : b0 + BC].transpose(1, 0, 2).reshape(L, FD)
        )
        cf = np.zeros((L, CF_W), np.float32)
        cf[:, 0:FD] = tr(T)
        cf[:, FD : FD + RC] = a[rl]
        cf[:, FD + RC : FD + 2 * RC] = nas[rl]
        cbf = np.zeros((L, CB_W), np.float32)
        cbf[:, 0:FD] = tr(X)
        cbf[:, FD : 2 * FD] = tr(M)
        cbf[:, 2 * FD : 3 * FD] = tr(PD)
        cbf[:, 3 * FD : 3 * FD + RC * RC] = oh.reshape(L, RC * RC)
        cbf[:, 3 * FD + RC * RC :] = ident
        wp = wts[rl].reshape(RC, L, 5 * D).transpose(1, 0, 2).reshape(L, RC * WS_W)
        btp = np.zeros((L, RC * L + 16), np.float32)
        for i, r in enumerate(rl):
            btp[0:D, i * L : (i + 1) * L] = bt4[r].T
            btp[0:D, RC * L + i] = lbv[r]
        in_maps.append(
            {
                "cf": cf,
                "cb": cbf.astype(_nbf16),
                "W": np.ascontiguousarray(wp).astype(_nbf16),
                "BT": btp.astype(_nbf16),
                "OHD": ohd.astype(_nbf16),
            }
        )
    return nfast, buckets, in_maps


def kernel(X, T, M, PD, alpha, w_v, w_t, b_t, b_v, ref_time):
    X = np.asarray(X, np.float32)
    T = np.asarray(T, np.float32)
    M = np.asarray(M, np.float32)
    PD = np.asarray(PD, np.float32)
    alpha = np.asarray(alpha, np.float32)
    w_v = np.asarray(w_v, np.float32)
    w_t = np.asarray(w_t, np.float32)
    b_t = np.asarray(b_t, np.float32)
    b_v = np.asarray(b_v, np.float32)
    ref_time = np.asarray(ref_time, np.float32)

    nfast, buckets, in_maps = _prepare(
        X, T, M, PD, alpha, w_v, w_t, b_t, b_v, ref_time
    )

    if nfast not in _CACHE:
        _CACHE[nfast] = _build_graph(nfast)
    nc = _CACHE[nfast]

    trace = bool(os.environ.get("BASS_KERNEL_TRACE"))
    kw = {}
    if trace:
        tmpdir = os.environ.get("BASS_KERNEL_TRACE_DIR") or None
        kw = dict(trace=True, tmpdir=tmpdir)
    res = run_bass_kernel_spmd(nc, in_maps, core_ids=list(range(8)), **kw)
    if trace:
        _CACHE["exec_time_ns"] = res.exec_time_ns
        print(f"HW exec time: {res.exec_time_ns} ns")

    out = np.zeros((B, R, D), np.float32)
    for c in range(8):
        b0 = (c // NK) * BC
        rl = buckets[c % NK]
        o = np.asarray(res.results[c]["out"], np.float32).reshape(RC, BC, D)
        for i, r in enumerate(rl):
            out[b0 : b0 + BC, r] = o[i]
    return out


# revision 12
# speedup vs baseline: 1.0770x; 1.0770x over previous
"""ALNN layer kernel for 8 TRN2 NeuronCores (raw Bass, explicit semaphores).

out[b,r,d] = relu( sum_l w_v[r,l,d]*relu(z[b,r,l,d]) + L*b_v[r,d] )
z = wt0*X + wt1*relu(X)*k + wt2*M + wt3*PD + 4*bt
k = exp(-relu(alpha_r)*|T - s_r|)        (uses relu(X*k) == relu(X)*k, k>0)

Sharding: B split 2 ways x R dealt into 4 buckets -> 8 cores, 16 b x 12 r
each. Cores c and c+4 share bucket c%4. r's with relu(alpha)=0 take a
fast path (k == 1: ABS/EXP/g skipped, q = relu(X)*wt1). The graph is
SPMD-uniform: a per-iteration fast-mask shared by all cores, sized by the
min zero-count across buckets (extra zeros run the slow path, still
correct since exp(-0*dist) == 1). Fast slots sit at the start (early DVE
start without waiting for ACT's k) and the last two slots (shorter drain).

Per-core layout: partitions = L(128), free = (b=16, d=64) = 1024.

v5 engine plan (evidence from v3/v4 traces: fixed ~6.5us framework
preamble + ~9us drain bracket the compute window; first DMA data lands
~8.7us; DVE busy ~40.5us is the roofline):
 - DVE: 4-5 weighted products per iter + wl (bf16 2x mode). Last two
   iterations' relu*wv fused as scalar_tensor_tensor from PSUM, the very
   last split in halves to pipeline the drain.
 - PE:  z in PSUM per iter: q/m0/m2/m3 identity matmuls (per-product
   vsem waits so the group starts as soon as q lands), then bt4
   reconstructed from a transposed 16KB copy via one-hot-d matmul
   (start-of-stream DMA stays lean); final L-reduction via one-hot
   columns; L*b_v added by the same one-hot-d trick right after rr=0.
 - ACT: dist=abs, k=exp (slow iters only, T in bf16), lat=relu(PSUM z)
   emitted relu-first each iteration, final relus straight from ps0/ps1,
   and the two output-half DMAs issued from ACT's own DGE ring.
 - DMA: descriptor generation parallelized across the two HWDGE rings
   (sync: X/W/M/PD/Ident stream; ACT: T/consts/OHD/BT), ordered to match
   first-use; BT/OHD ship only their 64 meaningful partitions.

Raw bass: this toolchain's walrus allows at most ONE attached sync-wait
per compute instruction, so cross-engine deps use standalone wait_ge
instructions; each DMA gets a dedicated semaphore (two DMAs sharing one
sem can interleave per-queue completions, so a partial wait would be
unsound).
"""

import os
import numpy as np
import ml_dtypes

import concourse.bass as bass
import concourse.mybir as mybir
from concourse.bass_utils import run_bass_kernel_spmd

AF = mybir.ActivationFunctionType
OP = mybir.AluOpType
BF16 = mybir.dt.bfloat16
F32 = mybir.dt.float32

B, R, L, D = 32, 48, 128, 64
NB, NK = 2, 4              # b-halves x r-buckets = 8 cores
BC, RC = B // NB, R // NK  # 16 b, 12 r per core
FD = BC * D                # 1024 free elems

CFC_W = 2 * RC             # f32 consts: [Abc 12 | NASbc 12]
# packed bf16 const layout: [Xt | Mt | PDt | OH 144 | I 128]
CB_W = 3 * FD + RC * RC + L
WS_W = 5 * D               # per-iter param slice: [wt0|wt1|wt2|wt3|wv]
BT_W = RC * L + 16         # bt4^T per iter + L*b_v^T columns

_nbf16 = ml_dtypes.bfloat16

LB = 5   # wl buffers (DVE -> PE)
LAB = 4  # lat buffers (ACT -> DVE)
PRB = 3  # product buffer sets (DVE -> PE)
ZB = 3   # psum z triple-buffer (6 of 8 banks; ps0/ps1 take the rest)
WLAG = 3  # wl(i-WLAG) emitted in DVE iter i
ALAG = 2  # lat(i-ALAG) emitted in ACT iter i
NWARM = 24  # PE warmup matmuls (keep PE out of low p-state until work)


def _fast_mask(nfast):
    """Fast slots first, plus the last two slots when available."""
    tail = min(2, nfast)
    lead = nfast - tail
    mask = [False] * RC
    for i in range(lead):
        mask[i] = True
    for i in range(RC - tail, RC):
        mask[i] = True
    return tuple(mask)


def _dve_schedule(mask):
    """DVE emission order; must match the @block.vector body exactly."""
    sched = [("xp", 0)]
    for i in range(RC):
        if i >= WLAG:
            sched.append(("wl", i - WLAG))
        if not mask[i]:
            sched.append(("g", i))
        sched.append(("q", i))
        sched.append(("m0", i))
        sched.append(("m2", i))
        sched.append(("m3", i))
    for rr in range(RC - WLAG, RC - 2):
        sched.append(("wl", rr))
    sched.append(("wlp", RC - 2))
    sched.append(("wlpa", RC - 1))
    sched.append(("wlpb", RC - 1))
    return sched


def _build_graph(mask, detect_races=True):
    nslow = sum(1 for f in mask if not f)
    ksl = max(nslow, 1)
    slows = [i for i in range(RC) if not mask[i]]
    kidx = {i: j for j, i in enumerate(slows)}  # slow iter -> k slot
    sched = _dve_schedule(mask)
    C = {key: idx + 1 for idx, key in enumerate(sched)}

    nc = bass.Bass(detect_race_conditions=detect_races)
    cfc_e = nc.declare_dram_parameter("cfc", [L, CFC_W], F32, isOutput=False)
    cft_e = nc.declare_dram_parameter("cfT", [L, FD], BF16, isOutput=False)
    cb_e = nc.declare_dram_parameter("cb", [L, CB_W], BF16, isOutput=False)
    W_e = nc.declare_dram_parameter("W", [L, RC * WS_W], BF16, isOutput=False)
    bt_e = nc.declare_dram_parameter("BT", [64, BT_W], BF16, isOutput=False)
    oh_e = nc.declare_dram_parameter("OHD", [64, 512], BF16, isOutput=False)
    out_e = nc.declare_dram_parameter("out", [RC, FD], F32, isOutput=True)

    from contextlib import ExitStack

    with ExitStack() as ctx:
        e = ctx.enter_context
        cfc = e(nc.sbuf_tensor([L, CFC_W], F32))
        cft = e(nc.sbuf_tensor([L, FD], BF16))
        cb = e(nc.sbuf_tensor([L, CB_W], BF16))
        Wb = e(nc.sbuf_tensor([L, RC * WS_W], BF16))
        BT = e(nc.sbuf_tensor([64, BT_W], BF16))
        OHD = e(nc.sbuf_tensor([64, 512], BF16))
        Xp = e(nc.sbuf_tensor([L, FD], BF16))
        dist = e(nc.sbuf_tensor([L, FD], F32))
        kbuf = e(nc.sbuf_tensor([L, ksl * FD], BF16))
        g = e(nc.sbuf_tensor([L, FD], BF16))
        qb = e(nc.sbuf_tensor([L, PRB * FD], BF16))
        m0b = e(nc.sbuf_tensor([L, PRB * FD], BF16))
        m2b = e(nc.sbuf_tensor([L, PRB * FD], BF16))
        m3b = e(nc.sbuf_tensor([L, PRB * FD], BF16))
        latb = e(nc.sbuf_tensor([L, LAB * FD], BF16))
        wlbuf = e(nc.sbuf_tensor([L, LB * FD], BF16))
        outt = e(nc.sbuf_tensor([RC, FD], F32))
        wsc = e(nc.sbuf_tensor([L, 512], BF16))
        psz = [e(nc.psum_tensor(f"psz{j}", [L, FD], F32)) for j in range(ZB)]
        ps0 = e(nc.psum_tensor([RC, 512], F32))
        ps1 = e(nc.psum_tensor([RC, 512], F32))
        cfcsem = e(nc.semaphore("cfcsem"))
        cftsem = e(nc.semaphore("cftsem"))
        cbsem = e(nc.semaphore("cbsem"))    # cb X part
        cbmsem = e(nc.semaphore("cbmsem"))  # cb M part
        cbpsem = e(nc.semaphore("cbpsem"))  # cb PD part
        cbtsem = e(nc.semaphore("cbtsem"))  # cb OH3+Ident part
        wsem0 = e(nc.semaphore("wsem0"))    # W iter 0
        wsem1 = e(nc.semaphore("wsem1"))    # W iter 1
        wsema = e(nc.semaphore("wsema"))    # W iters 2-5
        wsemb = e(nc.semaphore("wsemb"))    # W iters 6-11
        ohsem = e(nc.semaphore("ohsem"))    # OHD
        btsem = e(nc.semaphore("btsem"))    # BT
        asem = e(nc.semaphore("asem"))      # ACT k completions
        lsem = e(nc.semaphore("lsem"))      # ACT lat relu completions
        zsem = e(nc.semaphore("zsem"))      # PE z-group completions (1/iter)
        z2sem = e(nc.semaphore("z2sem"))    # PE last z-group half completions
        msem = e(nc.semaphore("msem"))      # PE out-mm completions
        vsem = e(nc.semaphore("vsem"))      # DVE op completions
        osem = e(nc.semaphore("osem"))
        gsem = e(nc.semaphore("gsem"))
        block = e(nc.Block())

        Abc = cfc[:, 0:RC]
        NASbc = cfc[:, RC : 2 * RC]
        Xt = cb[:, 0:FD]
        Mt = cb[:, FD : 2 * FD]
        PDt = cb[:, 2 * FD : 3 * FD]
        OH3 = cb[:, 3 * FD : 3 * FD + RC * RC].rearrange("p (r m) -> p r m", r=RC)
        Ident = cb[:, 3 * FD + RC * RC :]
        lbvT = BT[:, RC * L : RC * L + RC]

        def r3(ap):
            return ap.rearrange("p (b d) -> p b d", b=BC)

        def kslot(j):
            return kbuf[:, (j % ksl) * FD : (j % ksl + 1) * FD]

        def wbc(i, ei):
            base = i * WS_W + ei * D
            return Wb[:, base : base + D].unsqueeze(1).broadcast_to([L, BC, D])

        def wbch(i, ei):
            base = i * WS_W + ei * D
            return (
                Wb[:, base : base + D].unsqueeze(1).broadcast_to([L, BC // 2, D])
            )

        def btsl(i):
            return BT[:, i * L : (i + 1) * L]

        def latslot(rr):
            return latb[:, (rr % LAB) * FD : (rr % LAB + 1) * FD]

        def wlslot(rr):
            return wlbuf[:, (rr % LB) * FD : (rr % LB + 1) * FD]

        def prod(buf, i):
            return buf[:, (i % PRB) * FD : (i % PRB + 1) * FD]

        @block.sync
        def _(sp):
            sp.dma_start(out=cb[:, 0:FD], in_=cb_e[:, 0:FD]).then_inc(cbsem, 16)
            sp.dma_start(out=Wb[:, 0:WS_W], in_=W_e[:, 0:WS_W]).then_inc(wsem0, 16)
            sp.dma_start(out=cb[:, FD : 2 * FD], in_=cb_e[:, FD : 2 * FD]).then_inc(
                cbmsem, 16
            )
            sp.dma_start(
                out=Wb[:, WS_W : 2 * WS_W], in_=W_e[:, WS_W : 2 * WS_W]
            ).then_inc(wsem1, 16)
            sp.dma_start(
                out=cb[:, 2 * FD : 3 * FD], in_=cb_e[:, 2 * FD : 3 * FD]
            ).then_inc(cbpsem, 16)
            sp.dma_start(out=cb[:, 3 * FD :], in_=cb_e[:, 3 * FD :]).then_inc(
                cbtsem, 16
            )
            sp.dma_start(
                out=Wb[:, 2 * WS_W : 6 * WS_W], in_=W_e[:, 2 * WS_W : 6 * WS_W]
            ).then_inc(wsema, 16)
            sp.dma_start(
                out=Wb[:, 6 * WS_W :], in_=W_e[:, 6 * WS_W :]
            ).then_inc(wsemb, 16)

        @block.scalar
        def _(act):
            act.dma_start(out=cfc[:, :], in_=cfc_e[:, :]).then_inc(cfcsem, 16)
            act.dma_start(out=cft[:, :], in_=cft_e[:, :]).then_inc(cftsem, 16)
            act.dma_start(out=OHD[:, :], in_=oh_e[:, :]).then_inc(ohsem, 16)
            act.dma_start(out=BT[:, :], in_=bt_e[:, :]).then_inc(btsem, 16)
            act.wait_ge(cfcsem, 16)
            act.wait_ge(cftsem, 16)
            for i in range(RC):
                # relus rr=0..9 only; last two z's go through the DVE wlp path
                if ALAG <= i and i - ALAG <= RC - WLAG:
                    rr = i - ALAG
                    act.wait_ge(zsem, rr + 1)
                    if rr >= LAB:
                        act.wait_ge(vsem, C[("wl", rr - LAB)])
                    nc.scalar.activation(
                        latslot(rr), psz[rr % ZB][:, :], AF.Relu
                    ).then_inc(lsem, 1)
                if i < nslow:
                    si = slows[i]
                    nc.scalar.activation(
                        dist[:, :], cft[:, :], AF.Abs,
                        bias=NASbc[:, si : si + 1], scale=Abc[:, si : si + 1],
                    )
                    nc.scalar.activation(
                        kslot(i), dist[:, :], AF.Exp, scale=-1.0
                    ).then_inc(asem, 1)
            act.wait_ge(msem, 12)
            nc.scalar.activation(outt[:, 0:512], ps0[:, :], AF.Relu)
            act.dma_start(out=out_e[:, 0:512], in_=outt[:, 0:512]).then_inc(osem, 16)
            act.wait_ge(msem, 13)
            nc.scalar.activation(outt[:, 512:], ps1[:, :], AF.Relu)
            act.dma_start(out=out_e[:, 512:], in_=outt[:, 512:]).then_inc(osem, 16)

        @block.vector
        def _(ve):
            ve.wait_ge(cbsem, 16)
            nc.vector.tensor_scalar_max(Xp[:, :], Xt, 0.0).then_inc(vsem, 1)
            for i in range(RC):
                if i >= WLAG:
                    rr = i - WLAG
                    ve.wait_ge(lsem, rr + 1)
                    if rr >= LB:
                        ve.wait_ge(msem, rr - LB + 1)
                    nc.vector.tensor_tensor(
                        r3(wlslot(rr)), r3(latslot(rr)), wbc(rr, 4), OP.mult
                    ).then_inc(vsem, 1)
                if not mask[i]:
                    ve.wait_ge(asem, kidx[i] + 1)
                    nc.vector.tensor_mul(g[:, :], Xp[:, :], kslot(kidx[i])).then_inc(
                        vsem, 1
                    )
                    qsrc = g[:, :]
                else:
                    qsrc = Xp[:, :]
                if i == 0:
                    ve.wait_ge(wsem0, 16)
                elif i == 1:
                    ve.wait_ge(wsem1, 16)
                elif i == 2:
                    ve.wait_ge(wsema, 16)
                elif i == 6:
                    ve.wait_ge(wsemb, 16)
                nc.vector.tensor_tensor(
                    r3(prod(qb, i)), r3(qsrc), wbc(i, 1), OP.mult
                ).then_inc(vsem, 1)
                nc.vector.tensor_tensor(
                    r3(prod(m0b, i)), r3(Xt), wbc(i, 0), OP.mult
                ).then_inc(vsem, 1)
                if i == 0:
                    ve.wait_ge(cbmsem, 16)
                nc.vector.tensor_tensor(
                    r3(prod(m2b, i)), r3(Mt), wbc(i, 2), OP.mult
                ).then_inc(vsem, 1)
                if i == 0:
                    ve.wait_ge(cbpsem, 16)
                nc.vector.tensor_tensor(
                    r3(prod(m3b, i)), r3(PDt), wbc(i, 3), OP.mult
                ).then_inc(vsem, 1)
            for rr in range(RC - WLAG, RC - 2):
                ve.wait_ge(lsem, rr + 1)
                ve.wait_ge(msem, rr - LB + 1)
                nc.vector.tensor_tensor(
                    r3(wlslot(rr)), r3(latslot(rr)), wbc(rr, 4), OP.mult
                ).then_inc(vsem, 1)
            # fused relu*wv straight from PSUM for the last two iterations;
            # the very last one in halves so the out matmuls/relus pipeline
            rr = RC - 2
            ve.wait_ge(zsem, rr + 1)
            ve.wait_ge(msem, rr - LB + 1)
            nc.vector.scalar_tensor_tensor(
                r3(wlslot(rr)), r3(psz[rr % ZB][:, :]), 0.0, wbc(rr, 4),
                OP.max, OP.mult,
            ).then_inc(vsem, 1)
            rr = RC - 1
            ve.wait_ge(msem, rr - LB + 1)
            for h, zwait in ((0, 1), (1, 2)):
                ve.wait_ge(z2sem, zwait)
                c0, c1 = h * 512, (h + 1) * 512
                wl3 = wlslot(rr)[:, c0:c1].rearrange("p (b d) -> p b d", b=BC // 2)
                pz3 = psz[rr % ZB][:, c0:c1].rearrange(
                    "p (b d) -> p b d", b=BC // 2
                )
                nc.vector.scalar_tensor_tensor(
                    wl3, pz3, 0.0, wbch(rr, 4), OP.max, OP.mult
                ).then_inc(vsem, 1)

        @block.gpsimd
        def _(gp):
            nc.gpsimd.memset(wsc[:, :], 1.0).then_inc(gsem, 1)

        @block.tensor
        def _(te):
            # warmup: keep the PE out of its low p-state until real work
            # arrives. Results never read; ps0 reset by the real start=True.
            te.wait_ge(gsem, 1)
            for _w in range(NWARM):
                nc.tensor.matmul(
                    ps0[:, :], wsc[:, 0:RC], wsc[:, :],
                    start=True, stop=True, skip_group_check=True,
                )
            te.wait_ge(ohsem, 16)
            te.wait_ge(btsem, 16)
            te.wait_ge(cbtsem, 16)
            for i in range(RC):
                last = i == RC - 1
                if i >= ZB:
                    te.wait_ge(lsem, i - ZB + 1)
                pz = psz[i % ZB]
                prods = ((qb, "q"), (m0b, "m0"), (m2b, "m2"), (m3b, "m3"))
                if last:
                    # h0 stream first, then h1, each closed separately so the
                    # DVE's wlp halves overlap with this group's tail
                    for h in (0, 1):
                        for pb, tag in prods:
                            if h == 0:
                                te.wait_ge(vsem, C[(tag, i)])
                            c0, c1 = h * 512, (h + 1) * 512
                            nc.tensor.matmul(
                                pz[:, c0:c1], Ident, prod(pb, i)[:, c0:c1],
                                start=(tag == "q"), stop=False,
                                skip_group_check=True,
                            )
                        nc.tensor.matmul(
                            pz[:, h * 512 : (h + 1) * 512], btsl(i), OHD[:, :],
                            start=False, stop=True, skip_group_check=True,
                        ).then_inc(z2sem, 1)
                else:
                    for pb, tag in prods:
                        te.wait_ge(vsem, C[(tag, i)])
                        for h in (0, 1):
                            c0, c1 = h * 512, (h + 1) * 512
                            nc.tensor.matmul(
                                pz[:, c0:c1], Ident, prod(pb, i)[:, c0:c1],
                                start=(tag == "q"), stop=False,
                                skip_group_check=True,
                            )
                    for h in (0, 1):
                        mm = nc.tensor.matmul(
                            pz[:, h * 512 : (h + 1) * 512], btsl(i), OHD[:, :],
                            start=False, stop=True, skip_group_check=True,
                        )
                        if h == 1:
                            mm.then_inc(zsem, 1)
                if i >= WLAG:
                    rr = i - WLAG
                    te.wait_ge(vsem, C[("wl", rr)])
                    wl = wlslot(rr)
                    nc.tensor.matmul(
                        ps0[:, :], OH3[:, rr, :], wl[:, 0:512],
                        start=(rr == 0), stop=False, skip_group_check=True,
                    )
                    nc.tensor.matmul(
                        ps1[:, :], OH3[:, rr, :], wl[:, 512:1024],
                        start=(rr == 0), stop=False, skip_group_check=True,
                    ).then_inc(msem, 1)
                    if rr == 0:
                        # accumulate the L*b_v rows early (order irrelevant)
                        nc.tensor.matmul(
                            ps0[:, :], lbvT, OHD[:, 0:512],
                            start=False, stop=False, skip_group_check=True,
                        )
                        nc.tensor.matmul(
                            ps1[:, :], lbvT, OHD[:, 0:512],
                            start=False, stop=False, skip_group_check=True,
                        )
            for rr in range(RC - WLAG, RC - 1):
                key = ("wl", rr) if rr < RC - 2 else ("wlp", rr)
                te.wait_ge(vsem, C[key])
                wl = wlslot(rr)
                nc.tensor.matmul(
                    ps0[:, :], OH3[:, rr, :], wl[:, 0:512],
                    start=False, stop=False, skip_group_check=True,
                )
                nc.tensor.matmul(
                    ps1[:, :], OH3[:, rr, :], wl[:, 512:1024],
                    start=False, stop=False, skip_group_check=True,
                ).then_inc(msem, 1)
            rr = RC - 1
            wl = wlslot(rr)
            te.wait_ge(vsem, C[("wlpa", rr)])
            nc.tensor.matmul(
                ps0[:, :], OH3[:, rr, :], wl[:, 0:512],
                start=False, stop=True, skip_group_check=True,
            ).then_inc(msem, 1)
            te.wait_ge(vsem, C[("wlpb", rr)])
            nc.tensor.matmul(
                ps1[:, :], OH3[:, rr, :], wl[:, 512:1024],
                start=False, stop=True, skip_group_check=True,
            ).then_inc(msem, 1)

    return nc


_CACHE = {}


def _buckets(a):
    """Deal r-indices into NK buckets of RC; zeros occupy each bucket's
    fast-mask positions first. Returns (buckets, nfast)."""
    zeros = [r for r in range(R) if a[r] == 0.0]
    pos = [r for r in range(R) if a[r] != 0.0]
    zbuck = [[] for _ in range(NK)]
    for j, r in enumerate(zeros):
        zbuck[j % NK].append(r)
    nfast = min(min(len(zb) for zb in zbuck), RC)
    mask = _fast_mask(nfast)
    pi = 0
    buckets = []
    for k in range(NK):
        zq = list(zbuck[k])
        rl = [None] * RC
        for i in range(RC):
            if mask[i]:
                rl[i] = zq.pop(0)
        for i in range(RC):
            if rl[i] is None:
                if zq:
                    rl[i] = zq.pop(0)
                else:
                    rl[i] = pos[pi]
                    pi += 1
        buckets.append(rl)
    return buckets, nfast


def _prepare(X, T, M, PD, alpha, w_v, w_t, b_t, b_v, ref_time):
    """Pack full inputs into per-core DRAM parameter maps.
    Returns (mask, buckets, in_maps)."""
    a = np.maximum(alpha.reshape(R), 0.0)
    s_ref = ref_time.reshape(R)
    nas = -(a * s_ref)
    bt4 = 4.0 * b_t[..., 0]              # [R, L, D]
    lbv = float(L) * b_v[:, 0, :]        # [R, D]

    buckets, nfast = _buckets(a)
    mask = _fast_mask(nfast)

    # per-r params: [wt0|wt1|wt2|wt3|wv] (5*D per iter)
    wts = np.stack(
        [w_t[..., 0], w_t[..., 1], w_t[..., 2], w_t[..., 3], w_v], axis=2
    )                                     # [R, L, 5, D]

    oh = np.zeros((L, RC, RC), np.float32)
    for r in range(RC):
        oh[:, r, r] = 1.0
    ident = np.eye(L, dtype=np.float32)
    ohd = np.zeros((64, 512), np.float32)
    for b in range(8):
        for d in range(64):
            ohd[d, b * 64 + d] = 1.0

    in_maps = []
    for c in range(8):
        b0 = (c // NK) * BC
        rl = buckets[c % NK]
        tr = lambda x: np.ascontiguousarray(
            x[b0 : b0 + BC].transpose(1, 0, 2).reshape(L, FD)
        )
        cfc = np.zeros((L, CFC_W), np.float32)
        cfc[:, 0:RC] = a[rl]
        cfc[:, RC : 2 * RC] = nas[rl]
        cbf = np.zeros((L, CB_W), np.float32)
        cbf[:, 0:FD] = tr(X)
        cbf[:, FD : 2 * FD] = tr(M)
        cbf[:, 2 * FD : 3 * FD] = tr(PD)
        cbf[:, 3 * FD : 3 * FD + RC * RC] = oh.reshape(L, RC * RC)
        cbf[:, 3 * FD + RC * RC :] = ident
        wp = wts[rl].reshape(RC, L, 5 * D).transpose(1, 0, 2).reshape(L, RC * WS_W)
        btp = np.zeros((64, BT_W), np.float32)
        for i, r in enumerate(rl):
            btp[0:D, i * L : (i + 1) * L] = bt4[r].T
            btp[0:D, RC * L + i] = lbv[r]
        in_maps.append(
            {
                "cfc": cfc,
                "cfT": tr(T).astype(_nbf16),
                "cb": cbf.astype(_nbf16),
                "W": np.ascontiguousarray(wp).astype(_nbf16),
                "BT": btp.astype(_nbf16),
                "OHD": ohd.astype(_nbf16),
            }
        )
    return mask, buckets, in_maps


def kernel(X, T, M, PD, alpha, w_v, w_t, b_t, b_v, ref_time):
    X = np.asarray(X, np.float32)
    T = np.asarray(T, np.float32)
    M = np.asarray(M, np.float32)
    PD = np.asarray(PD, np.float32)
    alpha = np.asarray(alpha, np.float32)
    w_v = np.asarray(w_v, np.float32)
    w_t = np.asarray(w_t, np.float32)
    b_t = np.asarray(b_t, np.float32)
    b_v = np.asarray(b_v, np.float32)
    ref_time = np.asarray(ref_time, np.float32)

    mask, buckets, in_maps = _prepare(
        X, T, M, PD, alpha, w_v, w_t, b_t, b_v, ref_time
    )

    if mask not in _CACHE:
        _CACHE[mask] = _build_graph(mask)
    nc = _CACHE[mask]

    trace = bool(os.environ.get("BASS_KERNEL_TRACE"))
    kw = {}
    if trace:
        tmpdir = os.environ.get("BASS_KERNEL_TRACE_DIR") or None
        kw = dict(trace=True, tmpdir=tmpdir)
    res = run_bass_kernel_spmd(nc, in_maps, core_ids=list(range(8)), **kw)
    if trace:
        _CACHE["exec_time_ns"] = res.exec_time_ns
        print(f"HW exec time: {res.exec_time_ns} ns")

    out = np.zeros((B, R, D), np.float32)
    for c in range(8):
        b0 = (c // NK) * BC
        rl = buckets[c % NK]
        o = np.asarray(res.results[c]["out"], np.float32).reshape(RC, BC, D)
        for i, r in enumerate(rl):
            out[b0 : b0 + BC, r] = o[i]
    return out
